# revision 42
# baseline (speedup 1.0000x reference)
"""Trainium2 Bass kernel for nn_MinRNNPredictor (2-layer minGRU + FC head).

Sharding: data-parallel over batch — each of the 8 NeuronCores runs the
full network on one batch row (the recurrence is independent per row);
the small weight matrices are replicated. No collectives.

Per-core dataflow (all on-chip tensors in [feature, time] layout):
  x.T (bf16, cast + pre-transposed on host; plain contiguous DMA loads)
    -> GEMM0 (PE, bf16 in / fp32 PSUM): pre_z0, pre_h0  [H, Tc]
    -> gates (ScalarE sigmoid, DVE scalar_tensor_tensor)
    -> h0 via DVE TensorTensorScan along the free/time axis
    -> GEMM1 -> gates -> scan -> h1
    -> FC with h1 as the *stationary* operand, producing y in natural
       [time, feature] layout (no output transpose needed).

The layer-1 z-gate GEMM (the largest single GEMM) runs in fp8-e4m3 with
perf_mode=DoubleRow: Wz1 is host-quantized at x32 into a [P, K/256, 2, H]
pair layout, h0 gets an fp8 copy at x16 (one ScalarE Copy per tile), and
the 1/512 descale folds into the sigmoid's free scale parameter. Each
DoubleRow matmul contracts 256 rows (2 fp8 weights per PE cell), cutting
the z-GEMM's PE time roughly in half. Error budget: quantization noise on
pre_z1 only reaches h1 through the sigmoid (Lipschitz 1/4) and the scan's
EMA, simulated end-to-end at ~1.0e-2 vs the 2e-2 gate. The h-tilde paths
and the FC head stay bf16 - fp8 there lands directly on the output and
blows the budget.

Weights and x are cast to bf16/fp8 on the host once; biases are
pre-striped/broadcast on host so each is one clean DMA. Startup weight
DMAs are spread across the sync/vector/scalar queues (x on gpsimd) so the
first chunk's operands land in parallel instead of serializing behind one
queue.

The time axis is processed in chunks of 512 (one PSUM bank). The chunk
loop is software-pipelined: per iteration the PE runs GEMM0(i),
GEMM1(i-1) and FC(i-2), so the serial DVE scan chain of a chunk always
overlaps a full iteration of PE work instead of stalling the PE at
chunk boundaries.
"""

import os

# This kernel must run on the axon-tunneled NeuronCores. A host process may
# pin JAX_PLATFORMS=cpu for its own reference math; drop such a pin before
# jax is imported (via concourse) so jax.devices() still sees the cores.
_jp = os.environ.get("JAX_PLATFORMS")
if _jp is not None and "axon" not in _jp and "neuron" not in _jp:
    os.environ.pop("JAX_PLATFORMS", None)

from contextlib import ExitStack

import ml_dtypes
import numpy as np

import concourse.mybir as mybir
import concourse.tile as tile
from concourse import bacc, bass_utils

P = 128
B, T, DIN, H, DOUT = 8, 4096, 512, 1024, 512
TC = 512  # time-chunk = one PSUM bank of fp32

HS = 16.0  # fp8 storage scale for h0 (|16*h0| < ~80, e4m3 max 240)
AZ1 = 32.0  # fp8 storage scale for Wz1 (sigma 0.031 -> ~1.0)
SZ1 = HS * AZ1  # pre_z1 PSUM scale; descaled for free in the sigmoid
AZ0 = 16.0  # fp8 storage scale for Wz0 (sigma 0.044 -> 0.7); x stored at x1
SZ0 = AZ0  # pre_z0 PSUM scale

F32 = mybir.dt.float32
BF16 = mybir.dt.bfloat16
F8 = mybir.dt.float8e4
ALU = mybir.AluOpType
ACTF = mybir.ActivationFunctionType
DR = mybir.MatmulPerfMode.DoubleRow

WEIGHT_NAMES = ("Wz0", "bz0", "Wh0", "bh0", "Wz1", "bz1", "Wh1", "bh1", "Wfc", "bfc")


def build(t_total=T, tcc=TC):
    """Build the single-core Bass module (same NEFF runs SPMD on all cores)."""
    nchunk = t_total // tcc
    assert t_total % tcc == 0 and tcc % P == 0
    hsub = H // P

    nc = bacc.Bacc("TRN2", target_bir_lowering=False, debug=False, num_devices=B)
    # x pre-transposed on host to [DIN/P, P, T] so every [P, tcc] x.T tile
    # is a single clean contiguous-row DMA (no xbar transposes on chip).
    x_d = nc.dram_tensor("xT", [DIN // P, P, t_total], BF16, kind="ExternalInput").ap()
    # fp8 copy of x.T in DoubleRow pair layout ([J, p, j, t] = x[t, (2J+j)*128+p])
    # for the layer-0 z-gate GEMM; the h-path keeps the bf16 copy.
    x8_d = nc.dram_tensor(
        "x8T", [DIN // 256, P, 2, t_total], F8, kind="ExternalInput"
    ).ap()
    w_d = {}
    for name, shape, dt in (
        # Wz0 quantized on host at x16: [P, K/256, 2, H] fp8 pair layout.
        ("Wz0_8", [P, DIN // 256, 2, H], F8),
        ("Wh0", [DIN, H], BF16),
        # Wz1 quantized on host: [P, K/256, 2, H] fp8 pair layout for
        # DoubleRow ([p, J, j, n] = 32*Wz1[(2J+j)*128+p, n]).
        ("Wz1_8", [P, H // 256, 2, H], F8),
        ("Wh1", [H, H], BF16),
        ("Wfc", [H, DOUT], BF16),
        # Biases pre-striped on host: [P, 6*hsub] columns are
        # [bz0, bh0, bz1, bh1, -bz0, -bz1] stripes of [P, hsub] each.
        ("bias_pack", [P, 6 * (H // P)], F32),
        # FC bias pre-broadcast across partitions on host.
        ("bfc_rep", [P, DOUT], F32),
    ):
        w_d[name] = nc.dram_tensor(name, shape, dt, kind="ExternalInput").ap()
    y_d = nc.dram_tensor("y", [t_total, DOUT], F32, kind="ExternalOutput").ap()

    with tile.TileContext(nc) as tc, ExitStack() as ctx:
        const = ctx.enter_context(tc.tile_pool(name="const", bufs=1))
        sb = ctx.enter_context(tc.tile_pool(name="sb", bufs=2))
        psum = ctx.enter_context(tc.tile_pool(name="psum", bufs=8, space="PSUM"))

        xT_tiles = {}
        x8_tiles = {}
        h0_tiles = {}
        h8_tiles = {}
        h1_tiles = {}
        carry0 = [None] * hsub
        carry1 = [None] * hsub

        def emit_T(i):
            """Load the x.T tiles of chunk i (host pre-transposed). The fp8
            pair tiles go first: the z-gate DR matmuls consume them at the
            head of each chunk's m-loop."""
            x8 = []
            for J in range(DIN // 256):
                t_ = sb.tile([P, 2, tcc], F8, tag=f"x8_{J}", bufs=3, name=f"x8_{J}_{i}")
                nc.gpsimd.dma_start(t_[:], x8_d[J, :, :, i * tcc : (i + 1) * tcc])
                x8.append(t_)
            x8_tiles[i] = x8
            xT = []
            for dj in range(DIN // P):
                t_ = sb.tile([P, tcc], BF16, tag=f"xT{dj}", bufs=3, name=f"xT{dj}_{i}")
                nc.gpsimd.dma_start(t_[:], x_d[dj, :, i * tcc : (i + 1) * tcc])
                xT.append(t_)
            xT_tiles[i] = xT

        # PE warmup: the HAM clock gate holds the PE at half clock until it
        # has seen ~3.4us of sustained activity. The PE is idle waiting on
        # weight DMAs at kernel start anyway, so burn that window on zero
        # matmuls to arrive at the first real GEMM already at full clock.
        warm = const.tile([P, P], BF16, name="warm")
        nc.vector.memset(warm[:], 0.0)
        wp = psum.tile([P, P], F32, tag="psum", name="warm_psum")
        for _ in range(44):
            nc.tensor.matmul(wp[:], lhsT=warm[:], rhs=warm[:])

        # Bias pack first: one tiny clean DMA, needed by the first gates.
        bias_sb = const.tile([P, 6 * hsub], F32, name="bias_sb")
        nc.scalar.dma_start(bias_sb[:], w_d["bias_pack"])
        bz0_sb = bias_sb[:, 0 * hsub : 1 * hsub]
        bh0_sb = bias_sb[:, 1 * hsub : 2 * hsub]
        bz1_sb = bias_sb[:, 2 * hsub : 3 * hsub]
        bh1_sb = bias_sb[:, 3 * hsub : 4 * hsub]
        nbz0_sb = bias_sb[:, 4 * hsub : 5 * hsub]
        nbz1_sb = bias_sb[:, 5 * hsub : 6 * hsub]

        # Scratch target for the HBM-phase gate DMA (see below).
        gate_d = nc.dram_tensor("load_gate", [P, 4], BF16, kind="Internal").ap()

        # x chunks 0/1 next: chunk 0 gates the very first GEMM; chunk 1's
        # last tile doubles as the phase-gate source below.
        emit_T(0)
        emit_T(1)

        # Resident weights, contraction dim on partitions. Startup-critical
        # loads are spread across queues so they land in parallel: Wz0 on
        # sync, Wh0 on scalar (both feed GEMM0(0)), the rest stream behind.
        def load_w(name, k_dim, n_dim, eng, split=False):
            t_ = const.tile([P, k_dim // P, n_dim], BF16, name=f"{name}_sb")
            src = w_d[name].rearrange("(o p) n -> p o n", p=P)
            if split:
                # Per-k-tile DMAs: the first accumulation matmuls only wait
                # for their own k-slice instead of the whole weight.
                for k in range(k_dim // P):
                    eng.dma_start(t_[:, k : k + 1, :], src[:, k : k + 1, :])
            else:
                eng.dma_start(t_[:], src)
            return t_

        # fp8 Wz0 pair tiles, one DMA per 256-row group.
        wz0_sb = const.tile([P, DIN // 256, 2, H], F8, name="wz0_sb")
        for J in range(DIN // 256):
            nc.sync.dma_start(
                wz0_sb[:, J : J + 1, :, :], w_d["Wz0_8"][:, J : J + 1, :, :]
            )
        wh0_sb = load_w("Wh0", DIN, H, nc.scalar, split=True)
        # HBM is the head bottleneck (~8MB of loads competing at once, at
        # ~358GB/s). Gate the loads chunks 0/1 do NOT need behind a dummy
        # DMA that reads the last x tile of chunk 1: the sync queue stalls
        # there until the critical phase (Wz0_8/Wh0/x0/x1) has actually
        # landed, so the first two chunks' operands get the full HBM pipe.
        # Phase-2 weights then stream under chunk-0/1 compute, in
        # consumption order: Wh1 first (L1h(0) runs before L1z(0) now).
        nc.sync.dma_start(gate_d[:, :], xT_tiles[1][DIN // P - 1][:, 0:4])
        wh1_sb = load_w("Wh1", H, H, nc.sync, split=True)
        # fp8 Wz1 pair tiles, one DMA per 256-row group.
        wz1_sb = const.tile([P, H // 256, 2, H], F8, name="wz1_sb")
        for J in range(H // 256):
            nc.sync.dma_start(
                wz1_sb[:, J : J + 1, :, :], w_d["Wz1_8"][:, J : J + 1, :, :]
            )
        wfc_sb = load_w("Wfc", H, DOUT, nc.sync)

        # FC bias (pre-broadcast on host): one clean DMA, needed by FC(0)
        # ~100us in — emitted inside the loop to stay off the critical path.
        bfc_sb = const.tile([P, DOUT], F32, name="bfc_sb")

        def gates_scan0(i, m, pz, ph, outs):
            # a = 1 - z = sigmoid(-(pre_z + bz)); z = sigmoid(pre_z + bz)
            a_t = sb.tile([P, tcc], BF16, tag="a0", bufs=4, name=f"a0_{i}_{m}")
            nc.scalar.activation(
                a_t[:], pz[:], ACTF.Sigmoid, bias=nbz0_sb[:, m : m + 1],
                scale=-1.0 / SZ0,
            )
            z_t = sb.tile([P, tcc], BF16, tag="z0", bufs=4, name=f"z0_{i}_{m}")
            nc.scalar.activation(
                z_t[:], pz[:], ACTF.Sigmoid, bias=bz0_sb[:, m : m + 1],
                scale=1.0 / SZ0,
            )
            # b = (pre_h + bh) * z
            b_t = sb.tile([P, tcc], BF16, tag="b0", bufs=4, name=f"b0_{i}_{m}")
            nc.vector.scalar_tensor_tensor(
                b_t[:], ph[:], bh0_sb[:, m : m + 1], z_t[:], op0=ALU.add,
                op1=ALU.mult,
            )
            # h_t = a_t * h_{t-1} + b_t along the time (free) axis
            h_t = sb.tile([P, tcc], BF16, tag=f"h0_{m}", bufs=3, name=f"h0_{i}_{m}")
            init = 0.0 if carry0[m] is None else carry0[m][:, tcc - 1 : tcc]
            nc.vector.tensor_tensor_scan(
                h_t[:], a_t[:], b_t[:], init, op0=ALU.mult, op1=ALU.add
            )
            carry0[m] = h_t
            outs.append(h_t)

        def z0_mms(pz, m, rhs8):
            # z-path: fp8 DoubleRow, 256 contraction rows per matmul.
            for J in range(DIN // 256):
                nc.tensor.matmul(
                    pz[:],
                    lhsT=wz0_sb[:, J, :, m * P : (m + 1) * P],
                    rhs=rhs8[J][:],
                    start=(J == 0),
                    stop=(J == DIN // 256 - 1),
                    perf_mode=DR,
                )

        def emit_layer0(i):
            rhs_tiles = xT_tiles.pop(i)
            rhs8 = x8_tiles.pop(i)
            ksub = len(rhs_tiles)
            outs = []
            h8p = [
                sb.tile([P, 2, tcc], F8, tag=f"h8_{j}", bufs=3, name=f"h8_{j}_{i}")
                for j in range(hsub // 2)
            ]
            if i == 0:
                # Chunk 0 k-major in m-groups of 4 (8 PSUM banks): the PE
                # only waits for k-slice DMAs as they land (one slice per
                # queue cadence) instead of stalling on the full weight, so
                # the instruction stream is gapless from the first matmul
                # and the HAM clock gate ramps to full clock immediately.
                # Gates for the first group also issue ~4us earlier, hiding
                # the chunk-0 gate/scan chain under the group-1 matmuls.
                for mg in (range(0, 4), range(4, 8)):
                    pzs = {m: psum.tile([P, tcc], F32, tag="psum",
                                        name=f"pz0_{i}_{m}") for m in mg}
                    phs = {m: psum.tile([P, tcc], F32, tag="psum",
                                        name=f"ph0_{i}_{m}") for m in mg}
                    for m in mg:
                        z0_mms(pzs[m], m, rhs8)
                    for k in range(ksub):
                        for m in mg:
                            nc.tensor.matmul(
                                phs[m][:],
                                lhsT=wh0_sb[:, k, m * P : (m + 1) * P],
                                rhs=rhs_tiles[k][:],
                                start=(k == 0),
                                stop=(k == ksub - 1),
                            )
                    for m in mg:
                        gates_scan0(i, m, pzs[m], phs[m], outs)
            else:
                for m in range(hsub):
                    pz = psum.tile([P, tcc], F32, tag="psum", name=f"pz0_{i}_{m}")
                    ph = psum.tile([P, tcc], F32, tag="psum", name=f"ph0_{i}_{m}")
                    z0_mms(pz, m, rhs8)
                    for k in range(ksub):
                        nc.tensor.matmul(
                            ph[:],
                            lhsT=wh0_sb[:, k, m * P : (m + 1) * P],
                            rhs=rhs_tiles[k][:],
                            start=(k == 0),
                            stop=(k == ksub - 1),
                        )
                    gates_scan0(i, m, pz, ph, outs)
            # fp8 copies of h0 (x16) in DoubleRow pair layout for the L1
            # z-GEMM; emitted after the m-loop so they don't delay gates.
            for m in range(hsub):
                nc.scalar.activation(
                    h8p[m // 2][:, m % 2, :], outs[m][:], ACTF.Copy, bias=0.0,
                    scale=HS,
                )
            h0_tiles[i] = outs
            h8_tiles[i] = h8p

        def emit_layer1(i):
            rhs_tiles = h0_tiles.pop(i)
            h8p = h8_tiles.pop(i)
            ksub = len(rhs_tiles)
            outs = []
            for m in range(hsub):
                pz = psum.tile([P, tcc], F32, tag="psum", name=f"pz1_{i}_{m}")
                ph = psum.tile([P, tcc], F32, tag="psum", name=f"ph1_{i}_{m}")

                def z_mms():
                    # z-path: fp8 DoubleRow, 256 contraction rows per matmul.
                    for J in range(ksub // 2):
                        nc.tensor.matmul(
                            pz[:],
                            lhsT=wz1_sb[:, J, :, m * P : (m + 1) * P],
                            rhs=h8p[J][:],
                            start=(J == 0),
                            stop=(J == ksub // 2 - 1),
                            perf_mode=DR,
                        )

                def h_mms():
                    for k in range(ksub):
                        nc.tensor.matmul(
                            ph[:],
                            lhsT=wh1_sb[:, k, m * P : (m + 1) * P],
                            rhs=rhs_tiles[k][:],
                            start=(k == 0),
                            stop=(k == ksub - 1),
                        )

                # The h8 copies trail each chunk's bf16 scans by one ScalarE
                # pass; running the h-path (which needs only the bf16 scans)
                # first gives the copies a full h-GEMM of extra time.
                h_mms()
                z_mms()
                # pre_z1 arrives at scale SZ1; the sigmoid's input scale
                # undoes it for free. a = sigmoid(-(pre+bz)), z = sigmoid(+).
                a_t = sb.tile([P, tcc], BF16, tag="a1", bufs=4, name=f"a1_{i}_{m}")
                nc.scalar.activation(
                    a_t[:], pz[:], ACTF.Sigmoid, bias=nbz1_sb[:, m : m + 1],
                    scale=-1.0 / SZ1,
                )
                z_t = sb.tile([P, tcc], BF16, tag="z1", bufs=4, name=f"z1_{i}_{m}")
                nc.scalar.activation(
                    z_t[:], pz[:], ACTF.Sigmoid, bias=bz1_sb[:, m : m + 1],
                    scale=1.0 / SZ1,
                )
                b_t = sb.tile([P, tcc], BF16, tag="b1", bufs=4, name=f"b1_{i}_{m}")
                nc.vector.scalar_tensor_tensor(
                    b_t[:], ph[:], bh1_sb[:, m : m + 1], z_t[:], op0=ALU.add,
                    op1=ALU.mult,
                )
                h_t = sb.tile([P, tcc], BF16, tag=f"h1_{m}", bufs=3, name=f"h1_{i}_{m}")
                init = 0.0 if carry1[m] is None else carry1[m][:, tcc - 1 : tcc]
                nc.vector.tensor_tensor_scan(
                    h_t[:], a_t[:], b_t[:], init, op0=ALU.mult, op1=ALU.add
                )
                carry1[m] = h_t
                outs.append(h_t)
            h1_tiles[i] = outs

        def emit_FC(i):
            h1 = h1_tiles.pop(i)
            for tt in range(tcc // P):
                yp = psum.tile([P, DOUT], F32, tag="psum", name=f"yp_{i}_{tt}")
                for j in range(hsub):
                    nc.tensor.matmul(
                        yp[:],
                        lhsT=h1[j][:, tt * P : (tt + 1) * P],
                        rhs=wfc_sb[:, j, :],
                        start=(j == 0),
                        stop=(j == hsub - 1),
                    )
                y_sb = sb.tile([P, DOUT], F32, tag="y", bufs=4, name=f"y_{i}_{tt}")
                nc.vector.tensor_tensor(y_sb[:], yp[:], bfc_sb[:], ALU.add)
                t0 = i * tcc + tt * P
                nc.sync.dma_start(y_d[t0 : t0 + P, :], y_sb[:])

        # Software-pipelined chunk loop (stages offset on the PE stream).
        for i in range(nchunk + 2):
            if i < nchunk:
                emit_layer0(i)
            if i == 1:
                nc.scalar.dma_start(bfc_sb[:], w_d["bfc_rep"])
            if 1 <= i and i + 1 < nchunk:
                emit_T(i + 1)
            if 1 <= i <= nchunk:
                emit_layer1(i - 1)
            if 2 <= i <= nchunk + 1:
                emit_FC(i - 2)

    nc.compile()
    return nc


_NC_CACHE = {}


def _get_nc(t_total=T, tcc=TC):
    key = (t_total, tcc)
    if key not in _NC_CACHE:
        _NC_CACHE[key] = build(t_total, tcc)
    return _NC_CACHE[key]


_RUNNER = None


def _get_runner():
    """Build (once) a cached jitted SPMD executor for the module so repeated
    kernel() calls reuse the compiled NEFF instead of re-tracing."""
    global _RUNNER
    if _RUNNER is None:
        import jax
        from jax.experimental.shard_map import shard_map
        from jax.sharding import Mesh, PartitionSpec

        from concourse import bass2jax

        bass2jax.install_neuronx_cc_hook()
        nc = _get_nc()
        assert nc.dbg_addr is None
        partition_name = (
            nc.partition_id_tensor.name if nc.partition_id_tensor else None
        )
        in_names, out_names, out_avals = [], [], []
        for alloc in nc.m.functions[0].allocations:
            if not isinstance(alloc, mybir.MemoryLocationSet):
                continue
            name = alloc.memorylocations[0].name
            if alloc.kind == "ExternalInput":
                if name != partition_name:
                    in_names.append(name)
            elif alloc.kind == "ExternalOutput":
                out_names.append(name)
                out_avals.append(
                    jax.core.ShapedArray(
                        tuple(alloc.tensor_shape), mybir.dt.np(alloc.dtype)
                    )
                )
        n_params = len(in_names)
        n_outs = len(out_names)
        all_names = tuple(in_names) + tuple(out_names)
        if partition_name is not None:
            all_names = all_names + (partition_name,)

        def _body(*args):
            operands = list(args)
            if partition_name is not None:
                operands.append(bass2jax.partition_id_tensor())
            outs = bass2jax._bass_exec_p.bind(
                *operands,
                out_avals=tuple(out_avals),
                in_names=all_names,
                out_names=tuple(out_names),
                lowering_input_output_aliases=(),
                sim_require_finite=True,
                sim_require_nnan=True,
                nc=nc,
            )
            return tuple(outs)

        devices = jax.devices()[:B]
        assert len(devices) == B, f"need {B} cores, found {len(jax.devices())}"
        mesh = Mesh(np.asarray(devices), ("core",))
        sharded = jax.jit(
            shard_map(
                _body,
                mesh=mesh,
                in_specs=(PartitionSpec("core"),) * (n_params + n_outs),
                out_specs=(PartitionSpec("core"),) * n_outs,
                check_rep=False,
            ),
            donate_argnums=tuple(range(n_params, n_params + n_outs)),
            keep_unused=True,
        )
        _RUNNER = (sharded, list(in_names), list(out_names), list(out_avals))
    return _RUNNER


def pack_biases(inputs):
    """Host-side bias staging: stripe gate biases to [P, 6*hsub] (including
    negated z-biases) and broadcast bfc to [P, DOUT]."""
    hsub = H // P

    def stripe(name):
        return np.asarray(inputs[name], np.float32).reshape(hsub, P).T

    pack = np.concatenate(
        [
            stripe("bz0"), stripe("bh0"), stripe("bz1"), stripe("bh1"),
            -stripe("bz0"), -stripe("bz1"),
        ],
        axis=1,
    )
    bfc_rep = np.broadcast_to(
        np.asarray(inputs["bfc"], np.float32), (P, DOUT)
    )
    return {
        "bias_pack": np.ascontiguousarray(pack),
        "bfc_rep": np.ascontiguousarray(bfc_rep),
    }


def stage_weights(inputs):
    """Host-side weight staging: bf16 casts for the bf16 GEMMs, and scaled
    e4m3 DoubleRow pair layouts for the two z-gate weights."""
    shared = {}
    for name in ("Wh0", "Wh1", "Wfc"):
        shared[name] = np.ascontiguousarray(
            np.asarray(inputs[name], dtype=np.float32).astype(ml_dtypes.bfloat16)
        )

    def pack_dr(w, scale, k_dim):
        # k = (2J + j)*P + p  ->  [p, J, j, n]
        w8 = np.clip(np.asarray(w, np.float32) * scale, -240.0, 240.0)
        w8 = w8.astype(ml_dtypes.float8_e4m3)
        return np.ascontiguousarray(
            w8.reshape(k_dim // 256, 2, P, w8.shape[1]).transpose(2, 0, 1, 3)
        )

    shared["Wz0_8"] = pack_dr(inputs["Wz0"], AZ0, DIN)
    shared["Wz1_8"] = pack_dr(inputs["Wz1"], AZ1, H)
    shared.update(pack_biases(inputs))
    return shared


def run(inputs, trace=False, **spmd_kwargs):
    """Run the SPMD kernel on all 8 cores. Returns (y[B,T,DOUT], results)."""
    x = np.asarray(inputs["x"], dtype=np.float32)
    assert x.shape == (B, T, DIN), x.shape
    # [B, T, DIN] -> per-core [DIN/P, P, T] bf16 (cast + transpose staging)
    x_bf = np.ascontiguousarray(
        x.astype(ml_dtypes.bfloat16).transpose(0, 2, 1).reshape(B, DIN // P, P, T)
    )
    # fp8 copy in DR pair layout [J, p, j, t] (d = (2J+j)*P + p) for the
    # layer-0 z-gate GEMM.
    x8 = np.clip(x, -240.0, 240.0).astype(ml_dtypes.float8_e4m3)
    x8 = np.ascontiguousarray(
        x8.transpose(0, 2, 1)                      # [B, DIN, T]
        .reshape(B, DIN // 256, 2, P, T)
        .transpose(0, 1, 3, 2, 4)                  # [B, J, P, j, T]
    )
    shared = stage_weights(inputs)
    in_maps = [dict(shared, xT=x_bf[c], x8T=x8[c]) for c in range(B)]
    if trace or spmd_kwargs:
        nc = _get_nc()
        res = bass_utils.run_bass_kernel_spmd(
            nc, in_maps, core_ids=list(range(B)), trace=trace, **spmd_kwargs
        )
        y = np.stack([r["y"] for r in res.results], axis=0).astype(np.float32)
        return y, res
    sharded, in_names, out_names, out_avals = _get_runner()
    per_core = [[np.asarray(m[n]) for n in in_names] for m in in_maps]
    concat_in = [
        np.concatenate([per_core[c][i] for c in range(B)], axis=0)
        for i in range(len(in_names))
    ]
    concat_zeros = [
        np.zeros((B * a.shape[0], *a.shape[1:]), a.dtype) for a in out_avals
    ]
    outs = sharded(*concat_in, *concat_zeros)
    yi = out_names.index("y")
    y = np.asarray(outs[yi]).reshape(B, *out_avals[yi].shape).astype(np.float32)
    return y, None


def kernel(**inputs) -> np.ndarray:
    y, _ = run(inputs)
    return y


# revision 43
# speedup vs baseline: 1.0043x; 1.0043x over previous
"""Trainium2 Bass kernel for nn_MinRNNPredictor (2-layer minGRU + FC head).

Sharding: data-parallel over batch — each of the 8 NeuronCores runs the
full network on one batch row (the recurrence is independent per row);
the small weight matrices are replicated. No collectives.

Per-core dataflow (all on-chip tensors in [feature, time] layout):
  x.T (bf16, cast + pre-transposed on host; plain contiguous DMA loads)
    -> GEMM0 (PE, bf16 in / fp32 PSUM): pre_z0, pre_h0  [H, Tc]
    -> gates (ScalarE sigmoid, DVE scalar_tensor_tensor)
    -> h0 via DVE TensorTensorScan along the free/time axis
    -> GEMM1 -> gates -> scan -> h1
    -> FC with h1 as the *stationary* operand, producing y in natural
       [time, feature] layout (no output transpose needed).

The layer-1 z-gate GEMM (the largest single GEMM) runs in fp8-e4m3 with
perf_mode=DoubleRow: Wz1 is host-quantized at x32 into a [P, K/256, 2, H]
pair layout, h0 gets an fp8 copy at x16 (one ScalarE Copy per tile), and
the 1/512 descale folds into the sigmoid's free scale parameter. Each
DoubleRow matmul contracts 256 rows (2 fp8 weights per PE cell), cutting
the z-GEMM's PE time roughly in half. Error budget: quantization noise on
pre_z1 only reaches h1 through the sigmoid (Lipschitz 1/4) and the scan's
EMA, simulated end-to-end at ~1.0e-2 vs the 2e-2 gate. The h-tilde paths
and the FC head stay bf16 - fp8 there lands directly on the output and
blows the budget.

Weights and x are cast to bf16/fp8 on the host once; biases are
pre-striped/broadcast on host so each is one clean DMA. Startup weight
DMAs are spread across the sync/vector/scalar queues (x on gpsimd) so the
first chunk's operands land in parallel instead of serializing behind one
queue.

The time axis is processed in chunks of 512 (one PSUM bank). The chunk
loop is software-pipelined: per iteration the PE runs GEMM0(i),
GEMM1(i-1) and FC(i-2), so the serial DVE scan chain of a chunk always
overlaps a full iteration of PE work instead of stalling the PE at
chunk boundaries.
"""

import os

# This kernel must run on the axon-tunneled NeuronCores. A host process may
# pin JAX_PLATFORMS=cpu for its own reference math; drop such a pin before
# jax is imported (via concourse) so jax.devices() still sees the cores.
_jp = os.environ.get("JAX_PLATFORMS")
if _jp is not None and "axon" not in _jp and "neuron" not in _jp:
    os.environ.pop("JAX_PLATFORMS", None)

from contextlib import ExitStack

import ml_dtypes
import numpy as np

import concourse.mybir as mybir
import concourse.tile as tile
from concourse import bacc, bass_utils

P = 128
B, T, DIN, H, DOUT = 8, 4096, 512, 1024, 512
TC = 512  # time-chunk = one PSUM bank of fp32

HS = 16.0  # fp8 storage scale for h0 (|16*h0| < ~80, e4m3 max 240)
AZ1 = 32.0  # fp8 storage scale for Wz1 (sigma 0.031 -> ~1.0)
SZ1 = HS * AZ1  # pre_z1 PSUM scale; descaled for free in the sigmoid
AZ0 = 16.0  # fp8 storage scale for Wz0 (sigma 0.044 -> 0.7); x stored at x1
SZ0 = AZ0  # pre_z0 PSUM scale

F32 = mybir.dt.float32
BF16 = mybir.dt.bfloat16
F8 = mybir.dt.float8e4
ALU = mybir.AluOpType
ACTF = mybir.ActivationFunctionType
DR = mybir.MatmulPerfMode.DoubleRow

WEIGHT_NAMES = ("Wz0", "bz0", "Wh0", "bh0", "Wz1", "bz1", "Wh1", "bh1", "Wfc", "bfc")


def build(t_total=T, tcc=TC):
    """Build the single-core Bass module (same NEFF runs SPMD on all cores)."""
    nchunk = t_total // tcc
    assert t_total % tcc == 0 and tcc % P == 0
    hsub = H // P

    nc = bacc.Bacc("TRN2", target_bir_lowering=False, debug=False, num_devices=B)
    # x pre-transposed on host to [DIN/P, P, T] so every [P, tcc] x.T tile
    # is a single clean contiguous-row DMA (no xbar transposes on chip).
    x_d = nc.dram_tensor("xT", [DIN // P, P, t_total], BF16, kind="ExternalInput").ap()
    # fp8 copy of x.T in DoubleRow pair layout ([J, p, j, t] = x[t, (2J+j)*128+p])
    # for the layer-0 z-gate GEMM; the h-path keeps the bf16 copy.
    x8_d = nc.dram_tensor(
        "x8T", [DIN // 256, P, 2, t_total], F8, kind="ExternalInput"
    ).ap()
    w_d = {}
    for name, shape, dt in (
        # Wz0 quantized on host at x16: [P, K/256, 2, H] fp8 pair layout.
        ("Wz0_8", [P, DIN // 256, 2, H], F8),
        ("Wh0", [DIN, H], BF16),
        # Wz1 quantized on host: [P, K/256, 2, H] fp8 pair layout for
        # DoubleRow ([p, J, j, n] = 32*Wz1[(2J+j)*128+p, n]).
        ("Wz1_8", [P, H // 256, 2, H], F8),
        ("Wh1", [H, H], BF16),
        ("Wfc", [H, DOUT], BF16),
        # Biases pre-striped on host: [P, 6*hsub] columns are
        # [bz0, bh0, bz1, bh1, -bz0, -bz1] stripes of [P, hsub] each.
        ("bias_pack", [P, 6 * (H // P)], F32),
        # FC bias pre-broadcast across partitions on host.
        ("bfc_rep", [P, DOUT], F32),
    ):
        w_d[name] = nc.dram_tensor(name, shape, dt, kind="ExternalInput").ap()
    y_d = nc.dram_tensor("y", [t_total, DOUT], F32, kind="ExternalOutput").ap()

    with tile.TileContext(nc) as tc, ExitStack() as ctx:
        const = ctx.enter_context(tc.tile_pool(name="const", bufs=1))
        sb = ctx.enter_context(tc.tile_pool(name="sb", bufs=2))
        psum = ctx.enter_context(tc.tile_pool(name="psum", bufs=8, space="PSUM"))

        xT_tiles = {}
        x8_tiles = {}
        h0_tiles = {}
        h8_tiles = {}
        h1_tiles = {}
        carry0 = [None] * hsub
        carry1 = [None] * hsub

        def emit_T(i):
            """Load the x.T tiles of chunk i (host pre-transposed). The fp8
            pair tiles go first: the z-gate DR matmuls consume them at the
            head of each chunk's m-loop."""
            x8 = []
            for J in range(DIN // 256):
                t_ = sb.tile([P, 2, tcc], F8, tag=f"x8_{J}", bufs=3, name=f"x8_{J}_{i}")
                nc.gpsimd.dma_start(t_[:], x8_d[J, :, :, i * tcc : (i + 1) * tcc])
                x8.append(t_)
            x8_tiles[i] = x8
            xT = []
            for dj in range(DIN // P):
                t_ = sb.tile([P, tcc], BF16, tag=f"xT{dj}", bufs=3, name=f"xT{dj}_{i}")
                nc.gpsimd.dma_start(t_[:], x_d[dj, :, i * tcc : (i + 1) * tcc])
                xT.append(t_)
            xT_tiles[i] = xT

        # PE warmup: the HAM clock gate holds the PE at half clock until it
        # has seen ~3.4us of sustained activity. The PE is idle waiting on
        # weight DMAs at kernel start anyway, so burn that window on zero
        # matmuls to arrive at the first real GEMM already at full clock.
        warm = const.tile([P, P], BF16, name="warm")
        nc.vector.memset(warm[:], 0.0)
        wp = psum.tile([P, P], F32, tag="psum", name="warm_psum")
        for _ in range(28):
            nc.tensor.matmul(wp[:], lhsT=warm[:], rhs=warm[:])

        # Bias pack first: one tiny clean DMA, needed by the first gates.
        bias_sb = const.tile([P, 6 * hsub], F32, name="bias_sb")
        nc.scalar.dma_start(bias_sb[:], w_d["bias_pack"])
        bz0_sb = bias_sb[:, 0 * hsub : 1 * hsub]
        bh0_sb = bias_sb[:, 1 * hsub : 2 * hsub]
        bz1_sb = bias_sb[:, 2 * hsub : 3 * hsub]
        bh1_sb = bias_sb[:, 3 * hsub : 4 * hsub]
        nbz0_sb = bias_sb[:, 4 * hsub : 5 * hsub]
        nbz1_sb = bias_sb[:, 5 * hsub : 6 * hsub]

        # Scratch target for the HBM-phase gate DMA (see below).
        gate_d = nc.dram_tensor("load_gate", [P, 4], BF16, kind="Internal").ap()

        # x chunks 0/1 next: chunk 0 gates the very first GEMM; chunk 1's
        # last tile doubles as the phase-gate source below.
        emit_T(0)
        emit_T(1)

        # Resident weights, contraction dim on partitions. Startup-critical
        # loads are spread across queues so they land in parallel: Wz0 on
        # sync, Wh0 on scalar (both feed GEMM0(0)), the rest stream behind.
        def load_w(name, k_dim, n_dim, eng, split=False):
            t_ = const.tile([P, k_dim // P, n_dim], BF16, name=f"{name}_sb")
            src = w_d[name].rearrange("(o p) n -> p o n", p=P)
            if split:
                # Per-k-tile DMAs: the first accumulation matmuls only wait
                # for their own k-slice instead of the whole weight.
                for k in range(k_dim // P):
                    eng.dma_start(t_[:, k : k + 1, :], src[:, k : k + 1, :])
            else:
                eng.dma_start(t_[:], src)
            return t_

        # fp8 Wz0 pair tiles, one DMA per 256-row group.
        wz0_sb = const.tile([P, DIN // 256, 2, H], F8, name="wz0_sb")
        for J in range(DIN // 256):
            nc.sync.dma_start(
                wz0_sb[:, J : J + 1, :, :], w_d["Wz0_8"][:, J : J + 1, :, :]
            )
        wh0_sb = load_w("Wh0", DIN, H, nc.scalar, split=True)
        # HBM is the head bottleneck (~8MB of loads competing at once, at
        # ~358GB/s). Gate the loads chunks 0/1 do NOT need behind a dummy
        # DMA that reads the last x tile of chunk 1: the sync queue stalls
        # there until the critical phase (Wz0_8/Wh0/x0/x1) has actually
        # landed, so the first two chunks' operands get the full HBM pipe.
        # Phase-2 weights then stream under chunk-0/1 compute, in
        # consumption order: Wh1 first (L1h(0) runs before L1z(0) now).
        nc.sync.dma_start(gate_d[:, :], xT_tiles[1][DIN // P - 1][:, 0:4])
        wh1_sb = load_w("Wh1", H, H, nc.sync, split=True)
        # fp8 Wz1 pair tiles, one DMA per 256-row group.
        wz1_sb = const.tile([P, H // 256, 2, H], F8, name="wz1_sb")
        for J in range(H // 256):
            nc.sync.dma_start(
                wz1_sb[:, J : J + 1, :, :], w_d["Wz1_8"][:, J : J + 1, :, :]
            )
        wfc_sb = load_w("Wfc", H, DOUT, nc.sync)

        # FC bias (pre-broadcast on host): one clean DMA, needed by FC(0)
        # ~100us in — emitted inside the loop to stay off the critical path.
        bfc_sb = const.tile([P, DOUT], F32, name="bfc_sb")

        def gates_scan0(i, m, pz, ph, outs):
            # a = 1 - z = sigmoid(-(pre_z + bz)); z = sigmoid(pre_z + bz)
            a_t = sb.tile([P, tcc], BF16, tag="a0", bufs=4, name=f"a0_{i}_{m}")
            nc.scalar.activation(
                a_t[:], pz[:], ACTF.Sigmoid, bias=nbz0_sb[:, m : m + 1],
                scale=-1.0 / SZ0,
            )
            z_t = sb.tile([P, tcc], BF16, tag="z0", bufs=4, name=f"z0_{i}_{m}")
            nc.scalar.activation(
                z_t[:], pz[:], ACTF.Sigmoid, bias=bz0_sb[:, m : m + 1],
                scale=1.0 / SZ0,
            )
            # b = (pre_h + bh) * z
            b_t = sb.tile([P, tcc], BF16, tag="b0", bufs=4, name=f"b0_{i}_{m}")
            nc.vector.scalar_tensor_tensor(
                b_t[:], ph[:], bh0_sb[:, m : m + 1], z_t[:], op0=ALU.add,
                op1=ALU.mult,
            )
            # h_t = a_t * h_{t-1} + b_t along the time (free) axis
            h_t = sb.tile([P, tcc], BF16, tag=f"h0_{m}", bufs=3, name=f"h0_{i}_{m}")
            init = 0.0 if carry0[m] is None else carry0[m][:, tcc - 1 : tcc]
            nc.vector.tensor_tensor_scan(
                h_t[:], a_t[:], b_t[:], init, op0=ALU.mult, op1=ALU.add
            )
            carry0[m] = h_t
            outs.append(h_t)

        def z0_mms(pz, m, rhs8):
            # z-path: fp8 DoubleRow, 256 contraction rows per matmul.
            for J in range(DIN // 256):
                nc.tensor.matmul(
                    pz[:],
                    lhsT=wz0_sb[:, J, :, m * P : (m + 1) * P],
                    rhs=rhs8[J][:],
                    start=(J == 0),
                    stop=(J == DIN // 256 - 1),
                    perf_mode=DR,
                )

        def emit_layer0(i):
            rhs_tiles = xT_tiles.pop(i)
            rhs8 = x8_tiles.pop(i)
            ksub = len(rhs_tiles)
            outs = []
            h8p = [
                sb.tile([P, 2, tcc], F8, tag=f"h8_{j}", bufs=3, name=f"h8_{j}_{i}")
                for j in range(hsub // 2)
            ]
            if i == 0:
                # Chunk 0 k-major in m-groups of 4 (8 PSUM banks): the PE
                # only waits for k-slice DMAs as they land (one slice per
                # queue cadence) instead of stalling on the full weight, so
                # the instruction stream is gapless from the first matmul
                # and the HAM clock gate ramps to full clock immediately.
                # Gates for the first group also issue ~4us earlier, hiding
                # the chunk-0 gate/scan chain under the group-1 matmuls.
                for mg in (range(0, 4), range(4, 8)):
                    pzs = {m: psum.tile([P, tcc], F32, tag="psum",
                                        name=f"pz0_{i}_{m}") for m in mg}
                    phs = {m: psum.tile([P, tcc], F32, tag="psum",
                                        name=f"ph0_{i}_{m}") for m in mg}
                    for m in mg:
                        z0_mms(pzs[m], m, rhs8)
                    for k in range(ksub):
                        for m in mg:
                            nc.tensor.matmul(
                                phs[m][:],
                                lhsT=wh0_sb[:, k, m * P : (m + 1) * P],
                                rhs=rhs_tiles[k][:],
                                start=(k == 0),
                                stop=(k == ksub - 1),
                            )
                    for m in mg:
                        gates_scan0(i, m, pzs[m], phs[m], outs)
            else:
                for m in range(hsub):
                    pz = psum.tile([P, tcc], F32, tag="psum", name=f"pz0_{i}_{m}")
                    ph = psum.tile([P, tcc], F32, tag="psum", name=f"ph0_{i}_{m}")
                    z0_mms(pz, m, rhs8)
                    for k in range(ksub):
                        nc.tensor.matmul(
                            ph[:],
                            lhsT=wh0_sb[:, k, m * P : (m + 1) * P],
                            rhs=rhs_tiles[k][:],
                            start=(k == 0),
                            stop=(k == ksub - 1),
                        )
                    gates_scan0(i, m, pz, ph, outs)
            # fp8 copies of h0 (x16) in DoubleRow pair layout for the L1
            # z-GEMM; emitted after the m-loop so they don't delay gates.
            for m in range(hsub):
                nc.scalar.activation(
                    h8p[m // 2][:, m % 2, :], outs[m][:], ACTF.Copy, bias=0.0,
                    scale=HS,
                )
            h0_tiles[i] = outs
            h8_tiles[i] = h8p

        def emit_layer1(i):
            rhs_tiles = h0_tiles.pop(i)
            h8p = h8_tiles.pop(i)
            ksub = len(rhs_tiles)
            outs = []
            for m in range(hsub):
                pz = psum.tile([P, tcc], F32, tag="psum", name=f"pz1_{i}_{m}")
                ph = psum.tile([P, tcc], F32, tag="psum", name=f"ph1_{i}_{m}")

                def z_mms():
                    # z-path: fp8 DoubleRow, 256 contraction rows per matmul.
                    for J in range(ksub // 2):
                        nc.tensor.matmul(
                            pz[:],
                            lhsT=wz1_sb[:, J, :, m * P : (m + 1) * P],
                            rhs=h8p[J][:],
                            start=(J == 0),
                            stop=(J == ksub // 2 - 1),
                            perf_mode=DR,
                        )

                def h_mms():
                    for k in range(ksub):
                        nc.tensor.matmul(
                            ph[:],
                            lhsT=wh1_sb[:, k, m * P : (m + 1) * P],
                            rhs=rhs_tiles[k][:],
                            start=(k == 0),
                            stop=(k == ksub - 1),
                        )

                # The h8 copies trail each chunk's bf16 scans by one ScalarE
                # pass; running the h-path (which needs only the bf16 scans)
                # first gives the copies a full h-GEMM of extra time.
                h_mms()
                z_mms()
                # pre_z1 arrives at scale SZ1; the sigmoid's input scale
                # undoes it for free. a = sigmoid(-(pre+bz)), z = sigmoid(+).
                a_t = sb.tile([P, tcc], BF16, tag="a1", bufs=4, name=f"a1_{i}_{m}")
                nc.scalar.activation(
                    a_t[:], pz[:], ACTF.Sigmoid, bias=nbz1_sb[:, m : m + 1],
                    scale=-1.0 / SZ1,
                )
                z_t = sb.tile([P, tcc], BF16, tag="z1", bufs=4, name=f"z1_{i}_{m}")
                nc.scalar.activation(
                    z_t[:], pz[:], ACTF.Sigmoid, bias=bz1_sb[:, m : m + 1],
                    scale=1.0 / SZ1,
                )
                b_t = sb.tile([P, tcc], BF16, tag="b1", bufs=4, name=f"b1_{i}_{m}")
                nc.vector.scalar_tensor_tensor(
                    b_t[:], ph[:], bh1_sb[:, m : m + 1], z_t[:], op0=ALU.add,
                    op1=ALU.mult,
                )
                h_t = sb.tile([P, tcc], BF16, tag=f"h1_{m}", bufs=3, name=f"h1_{i}_{m}")
                init = 0.0 if carry1[m] is None else carry1[m][:, tcc - 1 : tcc]
                nc.vector.tensor_tensor_scan(
                    h_t[:], a_t[:], b_t[:], init, op0=ALU.mult, op1=ALU.add
                )
                carry1[m] = h_t
                outs.append(h_t)
            h1_tiles[i] = outs

        def emit_FC(i):
            h1 = h1_tiles.pop(i)
            for tt in range(tcc // P):
                yp = psum.tile([P, DOUT], F32, tag="psum", name=f"yp_{i}_{tt}")
                for j in range(hsub):
                    nc.tensor.matmul(
                        yp[:],
                        lhsT=h1[j][:, tt * P : (tt + 1) * P],
                        rhs=wfc_sb[:, j, :],
                        start=(j == 0),
                        stop=(j == hsub - 1),
                    )
                y_sb = sb.tile([P, DOUT], F32, tag="y", bufs=4, name=f"y_{i}_{tt}")
                nc.vector.tensor_tensor(y_sb[:], yp[:], bfc_sb[:], ALU.add)
                t0 = i * tcc + tt * P
                nc.sync.dma_start(y_d[t0 : t0 + P, :], y_sb[:])

        # Software-pipelined chunk loop (stages offset on the PE stream).
        for i in range(nchunk + 2):
            if i < nchunk:
                emit_layer0(i)
            if i == 1:
                nc.scalar.dma_start(bfc_sb[:], w_d["bfc_rep"])
            if 1 <= i and i + 1 < nchunk:
                emit_T(i + 1)
            if 1 <= i <= nchunk:
                emit_layer1(i - 1)
            if 2 <= i <= nchunk + 1:
                emit_FC(i - 2)

    nc.compile()
    return nc


_NC_CACHE = {}


def _get_nc(t_total=T, tcc=TC):
    key = (t_total, tcc)
    if key not in _NC_CACHE:
        _NC_CACHE[key] = build(t_total, tcc)
    return _NC_CACHE[key]


_RUNNER = None


def _get_runner():
    """Build (once) a cached jitted SPMD executor for the module so repeated
    kernel() calls reuse the compiled NEFF instead of re-tracing."""
    global _RUNNER
    if _RUNNER is None:
        import jax
        from jax.experimental.shard_map import shard_map
        from jax.sharding import Mesh, PartitionSpec

        from concourse import bass2jax

        bass2jax.install_neuronx_cc_hook()
        nc = _get_nc()
        assert nc.dbg_addr is None
        partition_name = (
            nc.partition_id_tensor.name if nc.partition_id_tensor else None
        )
        in_names, out_names, out_avals = [], [], []
        for alloc in nc.m.functions[0].allocations:
            if not isinstance(alloc, mybir.MemoryLocationSet):
                continue
            name = alloc.memorylocations[0].name
            if alloc.kind == "ExternalInput":
                if name != partition_name:
                    in_names.append(name)
            elif alloc.kind == "ExternalOutput":
                out_names.append(name)
                out_avals.append(
                    jax.core.ShapedArray(
                        tuple(alloc.tensor_shape), mybir.dt.np(alloc.dtype)
                    )
                )
        n_params = len(in_names)
        n_outs = len(out_names)
        all_names = tuple(in_names) + tuple(out_names)
        if partition_name is not None:
            all_names = all_names + (partition_name,)

        def _body(*args):
            operands = list(args)
            if partition_name is not None:
                operands.append(bass2jax.partition_id_tensor())
            outs = bass2jax._bass_exec_p.bind(
                *operands,
                out_avals=tuple(out_avals),
                in_names=all_names,
                out_names=tuple(out_names),
                lowering_input_output_aliases=(),
                sim_require_finite=True,
                sim_require_nnan=True,
                nc=nc,
            )
            return tuple(outs)

        devices = jax.devices()[:B]
        assert len(devices) == B, f"need {B} cores, found {len(jax.devices())}"
        mesh = Mesh(np.asarray(devices), ("core",))
        sharded = jax.jit(
            shard_map(
                _body,
                mesh=mesh,
                in_specs=(PartitionSpec("core"),) * (n_params + n_outs),
                out_specs=(PartitionSpec("core"),) * n_outs,
                check_rep=False,
            ),
            donate_argnums=tuple(range(n_params, n_params + n_outs)),
            keep_unused=True,
        )
        _RUNNER = (sharded, list(in_names), list(out_names), list(out_avals))
    return _RUNNER


def pack_biases(inputs):
    """Host-side bias staging: stripe gate biases to [P, 6*hsub] (including
    negated z-biases) and broadcast bfc to [P, DOUT]."""
    hsub = H // P

    def stripe(name):
        return np.asarray(inputs[name], np.float32).reshape(hsub, P).T

    pack = np.concatenate(
        [
            stripe("bz0"), stripe("bh0"), stripe("bz1"), stripe("bh1"),
            -stripe("bz0"), -stripe("bz1"),
        ],
        axis=1,
    )
    bfc_rep = np.broadcast_to(
        np.asarray(inputs["bfc"], np.float32), (P, DOUT)
    )
    return {
        "bias_pack": np.ascontiguousarray(pack),
        "bfc_rep": np.ascontiguousarray(bfc_rep),
    }


def stage_weights(inputs):
    """Host-side weight staging: bf16 casts for the bf16 GEMMs, and scaled
    e4m3 DoubleRow pair layouts for the two z-gate weights."""
    shared = {}
    for name in ("Wh0", "Wh1", "Wfc"):
        shared[name] = np.ascontiguousarray(
            np.asarray(inputs[name], dtype=np.float32).astype(ml_dtypes.bfloat16)
        )

    def pack_dr(w, scale, k_dim):
        # k = (2J + j)*P + p  ->  [p, J, j, n]
        w8 = np.clip(np.asarray(w, np.float32) * scale, -240.0, 240.0)
        w8 = w8.astype(ml_dtypes.float8_e4m3)
        return np.ascontiguousarray(
            w8.reshape(k_dim // 256, 2, P, w8.shape[1]).transpose(2, 0, 1, 3)
        )

    shared["Wz0_8"] = pack_dr(inputs["Wz0"], AZ0, DIN)
    shared["Wz1_8"] = pack_dr(inputs["Wz1"], AZ1, H)
    shared.update(pack_biases(inputs))
    return shared


def run(inputs, trace=False, **spmd_kwargs):
    """Run the SPMD kernel on all 8 cores. Returns (y[B,T,DOUT], results)."""
    x = np.asarray(inputs["x"], dtype=np.float32)
    assert x.shape == (B, T, DIN), x.shape
    # [B, T, DIN] -> per-core [DIN/P, P, T] bf16 (cast + transpose staging)
    x_bf = np.ascontiguousarray(
        x.astype(ml_dtypes.bfloat16).transpose(0, 2, 1).reshape(B, DIN // P, P, T)
    )
    # fp8 copy in DR pair layout [J, p, j, t] (d = (2J+j)*P + p) for the
    # layer-0 z-gate GEMM.
    x8 = np.clip(x, -240.0, 240.0).astype(ml_dtypes.float8_e4m3)
    x8 = np.ascontiguousarray(
        x8.transpose(0, 2, 1)                      # [B, DIN, T]
        .reshape(B, DIN // 256, 2, P, T)
        .transpose(0, 1, 3, 2, 4)                  # [B, J, P, j, T]
    )
    shared = stage_weights(inputs)
    in_maps = [dict(shared, xT=x_bf[c], x8T=x8[c]) for c in range(B)]
    if trace or spmd_kwargs:
        nc = _get_nc()
        res = bass_utils.run_bass_kernel_spmd(
            nc, in_maps, core_ids=list(range(B)), trace=trace, **spmd_kwargs
        )
        y = np.stack([r["y"] for r in res.results], axis=0).astype(np.float32)
        return y, res
    sharded, in_names, out_names, out_avals = _get_runner()
    per_core = [[np.asarray(m[n]) for n in in_names] for m in in_maps]
    concat_in = [
        np.concatenate([per_core[c][i] for c in range(B)], axis=0)
        for i in range(len(in_names))
    ]
    concat_zeros = [
        np.zeros((B * a.shape[0], *a.shape[1:]), a.dtype) for a in out_avals
    ]
    outs = sharded(*concat_in, *concat_zeros)
    yi = out_names.index("y")
    y = np.asarray(outs[yi]).reshape(B, *out_avals[yi].shape).astype(np.float32)
    return y, None


def kernel(**inputs) -> np.ndarray:
    y, _ = run(inputs)
    return y
